# Initial kernel scaffold
#
"""Trainium2 Bass kernel for a single-step attention GRU decoder.

Model (per reference):
    embedded = emb_table[x]                               # [B, E]
    energy   = tanh(enc @ W_w.T + W_b + (h @ U_w.T + U_b)[:, None, :])
    scores   = energy @ V_w[0] + V_b
    alpha    = softmax(scores, axis=S)
    context  = alpha @ enc                                # [B, E]
    GRU single step on [embedded, context] -> h_new       # [B, H]
    prediction = h_new @ fc_w.T + fc_b                    # [B, V]

Sharding (8 NeuronCores):
  - Attention + GRU are data-parallel over batch (8 rows/core).  The
    encoder slice is shipped pre-transposed ([E, B_loc*S]) so the
    contraction dim sits on SBUF partitions.
  - h_new^T shards are AllGathered on-chip (16 KB/core).
  - The fc layer is tensor-parallel over vocab: each core computes all
    64 batch rows against its 4000-vocab slice of fc_w; the host
    concatenates the logit shards.
  - The embedding gather (64 rows) plus weight transposes/sharding are
    host-side input prep; all FLOPs run on-device.  Matmuls use fp32r.
"""

import os
import sys

import numpy as np

if "/opt/trn_rl_repo" not in sys.path:
    sys.path.insert(0, "/opt/trn_rl_repo")

import concourse.bass as bass  # noqa: E402
import concourse.tile as tile  # noqa: E402
from concourse import bacc, mybir  # noqa: E402

F32 = mybir.dt.float32
F32R = mybir.dt.float32r
AF = mybir.ActivationFunctionType
OP = mybir.AluOpType

NCORES = 8
B, S, E, H, A, V = 64, 256, 512, 512, 512, 32000
BL = B // NCORES          # 8 batch rows per core
VS = V // NCORES          # 4000 vocab rows per core
R = BL * S                # 2048 attention rows per core
G3 = 3 * H                # 1536
NKT = E // 128            # 4 k-tiles per 512-dim contraction
RC = 512                  # row-chunk (free dim) for the energy matmul
NRC = R // RC             # 4 row chunks
BPC = RC // S             # 2 batch rows per row chunk
FCN = 500                 # fc free-dim chunk
NFC = VS // FCN           # 8 fc chunks


def _declare_io(nc):
    t = {}

    def inp(name, shape, dt=F32R):
        t[name] = nc.dram_tensor(name, list(shape), dt, kind="ExternalInput").ap()

    def outp(name, shape, dt=F32):
        t[name] = nc.dram_tensor(name, list(shape), dt, kind="ExternalOutput").ap()

    inp("encT", (E, R))            # encoder slice, transposed
    inp("wwT", (E, A))             # W_w.T
    inp("uwT", (H, A))             # U_w.T
    inp("vw", (1, A))              # V_w
    inp("wb", (1, A), F32)         # W_b
    inp("ub", (1, A), F32)         # U_b
    inp("vb", (1, 1), F32)         # V_b
    inp("hT", (H, BL))             # local hidden, transposed
    inp("hrow", (BL, H), F32)      # local hidden, row layout
    inp("embT", (E, BL))           # local embedded rows, transposed
    inp("wihT", (E + E, G3))       # W_ih.T  [1024, 1536]
    inp("whhT", (H, G3))           # W_hh.T
    inp("bih", (1, G3))            # b_ih
    inp("bhh", (1, G3))            # b_hh
    inp("fcT", (H, VS))            # local fc_w slice, transposed
    inp("fcb", (1, VS))            # local fc_b slice
    inp("ones", (1, B))            # ones for K=1 bias matmuls
    inp("id8", (BL, BL), F32)      # identity for PE transpose

    outp("logits", (B, VS))
    outp("hnew", (BL, H))

    # collective buffers
    t["cc_in"] = nc.dram_tensor("cc_in", [H, BL], F32).ap()
    t["cc_out"] = nc.dram_tensor(
        "cc_out", [NCORES, H, BL], F32, addr_space="Shared"
    ).ap()
    return t


def _emit(nc, tc, t):
    with (
        tc.tile_pool(name="persist", bufs=1) as pp,
        tc.tile_pool(name="wstream", bufs=12) as ws,
        tc.tile_pool(name="fcstream", bufs=14) as fs,
    ):
        # ---------------- persistent loads ----------------
        enc = []
        for k in range(NKT):
            e = pp.tile([128, R], F32R, tag=f"enc{k}")
            nc.sync.dma_start(out=e, in_=t["encT"][k * 128 : (k + 1) * 128, :])
            enc.append(e)
        ww, uw = [], []
        for k in range(NKT):
            w = pp.tile([128, A], F32R, tag=f"ww{k}")
            nc.sync.dma_start(out=w, in_=t["wwT"][k * 128 : (k + 1) * 128, :])
            ww.append(w)
            u = pp.tile([128, A], F32R, tag=f"uw{k}")
            nc.sync.dma_start(out=u, in_=t["uwT"][k * 128 : (k + 1) * 128, :])
            uw.append(u)
        vt = pp.tile([128, NKT], F32R, tag="vt")
        nc.sync.dma_start(out=vt, in_=t["vw"].rearrange("o (m p) -> (o p) m", p=128))
        hTt = pp.tile([128, NKT, BL], F32R, tag="hTt")
        nc.sync.dma_start(out=hTt, in_=t["hT"].rearrange("(k p) b -> p k b", p=128))
        embt = pp.tile([128, NKT, BL], F32R, tag="embt")
        nc.sync.dma_start(out=embt, in_=t["embT"].rearrange("(k p) b -> p k b", p=128))
        onest = pp.tile([1, B], F32R, tag="ones")
        nc.sync.dma_start(out=onest, in_=t["ones"])
        id8 = pp.tile([BL, BL], F32, tag="id8")
        nc.sync.dma_start(out=id8, in_=t["id8"])
        hrow = pp.tile([BL, H], F32, tag="hrow")
        nc.sync.dma_start(out=hrow, in_=t["hrow"])
        fcbt = pp.tile([1, VS], F32R, tag="fcbt")
        nc.sync.dma_start(out=fcbt, in_=t["fcb"])
        biht = pp.tile([1, G3], F32R, tag="biht")
        nc.sync.dma_start(out=biht, in_=t["bih"])
        bhht = pp.tile([1, G3], F32R, tag="bhht")
        nc.sync.dma_start(out=bhht, in_=t["bhh"])

        # bias(W_b + U_b + V_b is separate) per attention dim, [128, NKT]
        wbt = pp.tile([128, NKT], F32, tag="wbt")
        nc.sync.dma_start(out=wbt, in_=t["wb"].rearrange("o (m p) -> (o p) m", p=128))
        ubt = pp.tile([128, NKT], F32, tag="ubt")
        nc.sync.dma_start(out=ubt, in_=t["ub"].rearrange("o (m p) -> (o p) m", p=128))
        bwu = pp.tile([128, NKT], F32, tag="bwu")
        nc.vector.tensor_tensor(out=bwu, in0=wbt, in1=ubt, op=OP.add)
        vbt = pp.tile([1, 1], F32, tag="vbt")
        nc.sync.dma_start(out=vbt, in_=t["vb"])

        # ---------------- attention ----------------
        uh = pp.tile([128, NKT, BL], F32, tag="uh")  # U_w @ h.T + (W_b + U_b)
        sc = pp.tile([BL, S], F32, tag="sc")         # scores
        with (
            tc.tile_pool(name="psA", bufs=2, space="PSUM") as psA,
            tc.tile_pool(name="psE", bufs=3, space="PSUM") as psE,
            tc.tile_pool(name="psS", bufs=2, space="PSUM") as psS,
            tc.tile_pool(name="attn_sb", bufs=3) as asb,
        ):
            for m in range(NKT):
                pu = psA.tile([128, BL], F32, tag="pu")
                for k in range(NKT):
                    nc.tensor.matmul(
                        pu[:],
                        uw[k][:, m * 128 : (m + 1) * 128],
                        hTt[:, k, :],
                        start=(k == 0),
                        stop=(k == NKT - 1),
                    )
                nc.scalar.activation(
                    out=uh[:, m, :], in_=pu[:], func=AF.Copy,
                    bias=bwu[:, m : m + 1], scale=1.0,
                )

            for r in range(NRC):
                ps_s = psS.tile([1, RC], F32, tag="ps_s")
                for m in range(NKT):
                    pe = psE.tile([128, RC], F32, tag="pe")
                    for k in range(NKT):
                        nc.tensor.matmul(
                            pe[:],
                            ww[k][:, m * 128 : (m + 1) * 128],
                            enc[k][:, r * RC : (r + 1) * RC],
                            start=(k == 0),
                            stop=(k == NKT - 1),
                        )
                    et = asb.tile([128, RC], F32, tag="et")
                    nc.vector.tensor_tensor(
                        out=et[:].rearrange("p (b s) -> p b s", b=BPC),
                        in0=pe[:].rearrange("p (b s) -> p b s", b=BPC),
                        in1=uh[:, m, BPC * r : BPC * (r + 1)]
                        .unsqueeze(2)
                        .broadcast_to([128, BPC, S]),
                        op=OP.add,
                    )
                    tt = asb.tile([128, RC], F32R, tag="tt")
                    nc.scalar.activation(out=tt, in_=et, func=AF.Tanh)
                    nc.tensor.matmul(
                        ps_s[:],
                        vt[:, m : m + 1],
                        tt[:],
                        start=(m == 0),
                        stop=(m == NKT - 1),
                    )
                # scatter scores [1, 512] -> [2, 256] rows of sc
                nc.sync.dma_start(
                    out=sc[BPC * r : BPC * (r + 1), :],
                    in_=ps_s[:].rearrange("p (b s) -> p b s", b=BPC),
                )

        # softmax over S (alpha = softmax(sc + V_b) == softmax(sc))
        alpha = pp.tile([BL, S], F32, tag="alpha")
        with tc.tile_pool(name="soft", bufs=1) as sp:
            mx = sp.tile([BL, 1], F32, tag="mx")
            nc.vector.tensor_reduce(out=mx, in_=sc, axis=mybir.AxisListType.X, op=OP.max)
            mxn = sp.tile([BL, 1], F32, tag="mxn")
            nc.vector.tensor_scalar_mul(mxn[:], mx[:], -1.0)
            ex = sp.tile([BL, S], F32, tag="ex")
            sm = sp.tile([BL, 1], F32, tag="sm")
            nc.scalar.activation(
                out=ex, in_=sc, func=AF.Exp, bias=mxn[:], scale=1.0, accum_out=sm[:]
            )
            rs = sp.tile([BL, 1], F32, tag="rs")
            nc.vector.reciprocal(rs[:], sm[:])
            nc.vector.tensor_scalar_mul(alpha[:], ex[:], rs[:])

        # broadcast alpha to all partitions, then context^T via mul+reduce
        ctxT = pp.tile([128, NKT, BL], F32R, tag="ctxT")
        with tc.tile_pool(name="ctx_sb", bufs=2) as csb:
            ab = csb.tile([128, BL, S], F32, tag="ab")
            nc.gpsimd.dma_start(out=ab, in_=alpha[:].partition_broadcast(128))
            for k in range(NKT):
                mt = csb.tile([128, BL, S], F32, tag="mt")
                nc.vector.tensor_tensor(
                    out=mt,
                    in0=enc[k][:].bitcast(F32).rearrange("p (b s) -> p b s", b=BL),
                    in1=ab[:],
                    op=OP.mult,
                )
                nc.vector.tensor_reduce(
                    out=ctxT[:, k, :], in_=mt[:], axis=mybir.AxisListType.X, op=OP.add
                )

        # ---------------- GRU (local batch rows) ----------------
        hn_row = pp.tile([BL, H], F32, tag="hn_row")
        with (
            tc.tile_pool(name="psG", bufs=1, space="PSUM") as psG,
            tc.tile_pool(name="gru_sb", bufs=1) as gsb,
        ):
            def xcat(k):  # lhsT k-tile of [embedded; context]^T
                return embt[:, k, :] if k < NKT else ctxT[:, k - NKT, :]

            def gi_chunk(ps_tile, j, include_ih=True, include_hh=True):
                """Accumulate gate chunk j (cols j*512:(j+1)*512 of 3H)."""
                first = True
                if include_ih:
                    nc.tensor.matmul(
                        ps_tile[:], onest[:, :BL], biht[:, j * 512 : (j + 1) * 512],
                        start=first, stop=False,
                    )
                    first = False
                if include_hh:
                    nc.tensor.matmul(
                        ps_tile[:], onest[:, :BL], bhht[:, j * 512 : (j + 1) * 512],
                        start=first, stop=False,
                    )
                    first = False
                if include_ih:
                    for k in range(2 * NKT):
                        wt_ = ws.tile([128, 512], F32R, tag="wih")
                        nc.sync.dma_start(
                            out=wt_,
                            in_=t["wihT"][
                                k * 128 : (k + 1) * 128, j * 512 : (j + 1) * 512
                            ],
                        )
                        nc.tensor.matmul(
                            ps_tile[:], xcat(k), wt_[:],
                            start=False, stop=(not include_hh and k == 2 * NKT - 1),
                        )
                if include_hh:
                    for k in range(NKT):
                        wt_ = ws.tile([128, 512], F32R, tag="whh")
                        nc.sync.dma_start(
                            out=wt_,
                            in_=t["whhT"][
                                k * 128 : (k + 1) * 128, j * 512 : (j + 1) * 512
                            ],
                        )
                        nc.tensor.matmul(
                            ps_tile[:], hTt[:, k, :], wt_[:],
                            start=False, stop=(k == NKT - 1),
                        )

            prz0 = psG.tile([BL, 512], F32, tag="prz0")
            gi_chunk(prz0, 0)                          # r gate pre-activation
            prz1 = psG.tile([BL, 512], F32, tag="prz1")
            gi_chunk(prz1, 1)                          # z gate pre-activation
            pni = psG.tile([BL, 512], F32, tag="pni")
            gi_chunk(pni, 2, include_hh=False)         # i_n
            pnh = psG.tile([BL, 512], F32, tag="pnh")
            gi_chunk(pnh, 2, include_ih=False)         # h_n

            rg = gsb.tile([BL, 512], F32, tag="rg")
            nc.scalar.activation(out=rg, in_=prz0[:], func=AF.Sigmoid)
            zg = gsb.tile([BL, 512], F32, tag="zg")
            nc.scalar.activation(out=zg, in_=prz1[:], func=AF.Sigmoid)
            t1 = gsb.tile([BL, 512], F32, tag="t1")
            nc.vector.tensor_tensor(out=t1, in0=rg, in1=pnh[:], op=OP.mult)
            t2 = gsb.tile([BL, 512], F32, tag="t2")
            nc.vector.tensor_tensor(out=t2, in0=t1, in1=pni[:], op=OP.add)
            ng = gsb.tile([BL, 512], F32, tag="ng")
            nc.scalar.activation(out=ng, in_=t2, func=AF.Tanh)
            # h_new = n + z * (h - n)
            dg = gsb.tile([BL, 512], F32, tag="dg")
            nc.vector.tensor_tensor(out=dg, in0=hrow, in1=ng, op=OP.subtract)
            zd = gsb.tile([BL, 512], F32, tag="zd")
            nc.vector.tensor_tensor(out=zd, in0=zg, in1=dg, op=OP.mult)
            nc.vector.tensor_tensor(out=hn_row, in0=ng, in1=zd, op=OP.add)

        nc.sync.dma_start(out=t["hnew"], in_=hn_row[:])

        # transpose h_new -> [H, BL] and AllGather across cores
        hnT = pp.tile([128, NKT, BL], F32, tag="hnT")
        with tc.tile_pool(name="psT", bufs=2, space="PSUM") as psT:
            for k in range(NKT):
                ptr = psT.tile([128, BL], F32, tag="ptr")
                nc.tensor.transpose(
                    ptr[:], hn_row[:, k * 128 : (k + 1) * 128], id8[:]
                )
                nc.scalar.copy(out=hnT[:, k, :], in_=ptr[:])
        nc.sync.dma_start(
            out=t["cc_in"].rearrange("(k p) b -> p k b", p=128), in_=hnT[:]
        )
        nc.gpsimd.collective_compute(
            "AllGather",
            OP.bypass,
            replica_groups=[list(range(NCORES))],
            ins=[t["cc_in"]],
            outs=[t["cc_out"]],
        )

        # ---------------- fc (vocab shard, full batch) ----------------
        with (
            tc.tile_pool(name="psF", bufs=3, space="PSUM") as psF,
            tc.tile_pool(name="fc_sb", bufs=1) as fsb,
        ):
            hTf = []
            for k in range(NKT):
                ht = fsb.tile([128, B], F32R, tag=f"hTf{k}")
                nc.gpsimd.dma_start(
                    out=ht,
                    in_=t["cc_out"].rearrange("c (k p) b -> k p c b", p=128)[k],
                )
                hTf.append(ht)
            for j in range(NFC):
                pf = psF.tile([B, FCN], F32, tag="pf")
                nc.tensor.matmul(
                    pf[:], onest[:, :B], fcbt[:, j * FCN : (j + 1) * FCN],
                    start=True, stop=False,
                )
                for k in range(NKT):
                    ft = fs.tile([128, FCN], F32R, tag="fct")
                    nc.sync.dma_start(
                        out=ft,
                        in_=t["fcT"][k * 128 : (k + 1) * 128, j * FCN : (j + 1) * FCN],
                    )
                    nc.tensor.matmul(
                        pf[:], hTf[k][:], ft[:], start=False, stop=(k == NKT - 1)
                    )
                nc.sync.dma_start(
                    out=t["logits"][:, j * FCN : (j + 1) * FCN], in_=pf[:]
                )


def build_program():
    nc = bacc.Bacc(
        "TRN2", target_bir_lowering=False, debug=False, num_devices=NCORES
    )
    t = _declare_io(nc)
    with tile.TileContext(nc) as tc:
        _emit(nc, tc, t)
    nc.compile()
    return nc


def prepare_in_maps(inputs):
    """Shard + lay out the full inputs into 8 per-core input dicts."""
    f = np.float32
    x = np.asarray(inputs["x"])
    h = np.asarray(inputs["hidden"], dtype=f)[0]              # [B, H]
    enc = np.asarray(inputs["encoder_outputs"], dtype=f)      # [B, S, E]
    embedded = np.asarray(inputs["emb_table"], dtype=f)[x]    # [B, E]

    def C(a):
        return np.ascontiguousarray(a, dtype=f)

    shared = {
        "wwT": C(np.asarray(inputs["W_w"]).T),
        "uwT": C(np.asarray(inputs["U_w"]).T),
        "vw": C(np.asarray(inputs["V_w"]).reshape(1, A)),
        "wb": C(np.asarray(inputs["W_b"]).reshape(1, A)),
        "ub": C(np.asarray(inputs["U_b"]).reshape(1, A)),
        "vb": C(np.asarray(inputs["V_b"]).reshape(1, 1)),
        "wihT": C(np.asarray(inputs["W_ih"]).T),
        "whhT": C(np.asarray(inputs["W_hh"]).T),
        "bih": C(np.asarray(inputs["b_ih"]).reshape(1, G3)),
        "bhh": C(np.asarray(inputs["b_hh"]).reshape(1, G3)),
        "ones": np.ones((1, B), dtype=f),
        "id8": np.eye(BL, dtype=f),
    }
    fc_w = np.asarray(inputs["fc_w"], dtype=f)
    fc_b = np.asarray(inputs["fc_b"], dtype=f)

    in_maps = []
    for c in range(NCORES):
        bs = slice(c * BL, (c + 1) * BL)
        vs = slice(c * VS, (c + 1) * VS)
        m = dict(shared)
        m["encT"] = C(enc[bs].reshape(R, E).T)
        m["hT"] = C(h[bs].T)
        m["hrow"] = C(h[bs])
        m["embT"] = C(embedded[bs].T)
        m["fcT"] = C(fc_w[vs].T)
        m["fcb"] = C(fc_b[vs].reshape(1, VS))
        in_maps.append(m)
    return in_maps


def assemble(results):
    logits = np.concatenate([results[c]["logits"] for c in range(NCORES)], axis=1)
    hnew = np.concatenate([results[c]["hnew"] for c in range(NCORES)], axis=0)
    return logits, hnew[None]


_CACHED_NC = None


def kernel(**inputs):
    global _CACHED_NC
    from concourse.bass_utils import run_bass_kernel_spmd

    if _CACHED_NC is None:
        _CACHED_NC = build_program()
    in_maps = prepare_in_maps(inputs)
    res = run_bass_kernel_spmd(_CACHED_NC, in_maps, list(range(NCORES)))
    return assemble(res.results)


# revision 21
# speedup vs baseline: 1.2182x; 1.2182x over previous
"""Trainium2 Bass kernel for a single-step attention GRU decoder.

Model (per reference):
    embedded = emb_table[x]                               # [B, E]
    energy   = tanh(enc @ W_w.T + W_b + (h @ U_w.T + U_b)[:, None, :])
    scores   = energy @ V_w[0] + V_b
    alpha    = softmax(scores, axis=S)
    context  = alpha @ enc                                # [B, E]
    GRU single step on [embedded, context] -> h_new       # [B, H]
    prediction = h_new @ fc_w.T + fc_b                    # [B, V]

Sharding (8 NeuronCores):
  - Attention + GRU are data-parallel over batch (8 rows/core).  The
    encoder slice is shipped pre-transposed ([E, B_loc*S]) so the
    contraction dim sits on SBUF partitions.
  - h_new^T shards are AllGathered on-chip (16 KB/core).
  - The fc layer is tensor-parallel over vocab: each core computes all
    64 batch rows against its 4000-vocab slice of fc_w; the host
    concatenates the logit shards.
  - The embedding gather (64 rows) plus weight transposes/sharding are
    host-side input prep; all FLOPs run on-device.  Matmuls use fp32r.
"""

import os
import sys

import numpy as np

if "/opt/trn_rl_repo" not in sys.path:
    sys.path.insert(0, "/opt/trn_rl_repo")

import concourse.bass as bass  # noqa: E402
import concourse.tile as tile  # noqa: E402
from concourse import bacc, mybir  # noqa: E402

F32 = mybir.dt.float32
F32R = mybir.dt.float32r
AF = mybir.ActivationFunctionType
OP = mybir.AluOpType

NCORES = 8
B, S, E, H, A, V = 64, 256, 512, 512, 512, 32000
BL = B // NCORES          # 8 batch rows per core
VS = V // NCORES          # 4000 vocab rows per core
R = BL * S                # 2048 attention rows per core
G3 = 3 * H                # 1536
NKT = E // 128            # 4 k-tiles per 512-dim contraction
RC = 512                  # row-chunk (free dim) for the energy matmul
NRC = R // RC             # 4 row chunks
BPC = RC // S             # 2 batch rows per row chunk
FCN = 500                 # fc free-dim chunk
NFC = VS // FCN           # 8 fc chunks


def _declare_io(nc):
    t = {}

    def inp(name, shape, dt=F32R):
        t[name] = nc.dram_tensor(name, list(shape), dt, kind="ExternalInput").ap()

    def outp(name, shape, dt=F32):
        t[name] = nc.dram_tensor(name, list(shape), dt, kind="ExternalOutput").ap()

    inp("encT", (E, R))            # encoder slice, transposed
    inp("wwT", (E, A))             # W_w.T
    inp("uwT", (H, A))             # U_w.T
    inp("vw", (1, A))              # V_w
    inp("wb", (1, A), F32)         # W_b
    inp("ub", (1, A), F32)         # U_b
    inp("hT", (H, BL))             # local hidden, transposed
    inp("hrow", (BL, H), F32)      # local hidden, row layout
    inp("embT", (E, BL))           # local embedded rows, transposed
    inp("wihT", (E + E, G3))       # W_ih.T  [1024, 1536]
    inp("whhT", (H, G3))           # W_hh.T
    inp("bih", (1, G3))            # b_ih
    inp("bhh", (1, G3))            # b_hh
    inp("fcT", (H, VS))            # local fc_w slice, transposed
    inp("fcb", (1, VS))            # local fc_b slice
    inp("ones", (1, B))            # ones for K=1 bias matmuls
    inp("id8", (BL, BL), F32)      # identity for PE transpose

    outp("logits", (B, VS))
    outp("hnew", (BL, H))

    # scratch + collective buffers
    t["alpha_d"] = nc.dram_tensor("alpha_d", [BL, S], F32).ap()
    t["cc_in"] = nc.dram_tensor("cc_in", [H, BL], F32).ap()
    t["cc_out"] = nc.dram_tensor(
        "cc_out", [NCORES, H, BL], F32, addr_space="Shared"
    ).ap()
    return t


def _emit(nc, tc, t):
    with (
        tc.tile_pool(name="persist", bufs=1) as pp,
        tc.tile_pool(name="wstream", bufs=8) as ws,
        tc.tile_pool(name="fcstream", bufs=12) as fs,
        tc.tile_pool(name="bias1", bufs=2) as bs,
    ):
        # ---------------- persistent loads ----------------
        enc = []
        for k in range(NKT):
            e = pp.tile([128, R], F32R, tag=f"enc{k}")
            nc.sync.dma_start(out=e, in_=t["encT"][k * 128 : (k + 1) * 128, :])
            enc.append(e)
        ww, uw = [], []
        for k in range(NKT):
            w = pp.tile([128, A], F32R, tag=f"ww{k}")
            nc.sync.dma_start(out=w, in_=t["wwT"][k * 128 : (k + 1) * 128, :])
            ww.append(w)
            u = pp.tile([128, A], F32R, tag=f"uw{k}")
            nc.sync.dma_start(out=u, in_=t["uwT"][k * 128 : (k + 1) * 128, :])
            uw.append(u)
        vt = pp.tile([128, NKT], F32R, tag="vt")
        nc.sync.dma_start(out=vt, in_=t["vw"].rearrange("o (m p) -> (o p) m", p=128))
        hTt = pp.tile([128, NKT, BL], F32R, tag="hTt")
        nc.sync.dma_start(out=hTt, in_=t["hT"].rearrange("(k p) b -> p k b", p=128))
        embt = pp.tile([128, NKT, BL], F32R, tag="embt")
        nc.sync.dma_start(out=embt, in_=t["embT"].rearrange("(k p) b -> p k b", p=128))
        onest = pp.tile([1, B], F32R, tag="ones")
        nc.sync.dma_start(out=onest, in_=t["ones"])
        id8 = pp.tile([BL, BL], F32, tag="id8")
        nc.sync.dma_start(out=id8, in_=t["id8"])
        hrow = pp.tile([BL, H], F32, tag="hrow")
        nc.sync.dma_start(out=hrow, in_=t["hrow"])

        # bias(W_b + U_b + V_b is separate) per attention dim, [128, NKT]
        wbt = pp.tile([128, NKT], F32, tag="wbt")
        nc.sync.dma_start(out=wbt, in_=t["wb"].rearrange("o (m p) -> (o p) m", p=128))
        ubt = pp.tile([128, NKT], F32, tag="ubt")
        nc.sync.dma_start(out=ubt, in_=t["ub"].rearrange("o (m p) -> (o p) m", p=128))
        bwu = pp.tile([128, NKT], F32, tag="bwu")
        nc.vector.tensor_tensor(out=bwu, in0=wbt, in1=ubt, op=OP.add)
        # V_b shifts every score equally -> softmax-invariant; not loaded.

        # ---------------- attention ----------------
        uh = pp.tile([128, NKT, BL], F32, tag="uh")  # U_w @ h.T + (W_b + U_b)
        sc = pp.tile([BL, S], F32, tag="sc")         # scores
        with (
            tc.tile_pool(name="psA", bufs=2, space="PSUM") as psA,
            tc.tile_pool(name="psE", bufs=3, space="PSUM") as psE,
            tc.tile_pool(name="psS", bufs=2, space="PSUM") as psS,
            tc.tile_pool(name="attn_sb", bufs=3) as asb,
        ):
            for m in range(NKT):
                pu = psA.tile([128, BL], F32, tag="pu")
                for k in range(NKT):
                    nc.tensor.matmul(
                        pu[:],
                        uw[k][:, m * 128 : (m + 1) * 128],
                        hTt[:, k, :],
                        start=(k == 0),
                        stop=(k == NKT - 1),
                    )
                nc.vector.tensor_scalar_add(uh[:, m, :], pu[:], bwu[:, m : m + 1])

            for r in range(NRC):
                ps_s = psS.tile([1, RC], F32, tag="ps_s")
                for m in range(NKT):
                    pe = psE.tile([128, RC], F32, tag="pe")
                    for k in range(NKT):
                        nc.tensor.matmul(
                            pe[:],
                            ww[k][:, m * 128 : (m + 1) * 128],
                            enc[k][:, r * RC : (r + 1) * RC],
                            start=(k == 0),
                            stop=(k == NKT - 1),
                        )
                    et = asb.tile([128, RC], F32, tag="et")
                    nc.vector.tensor_tensor(
                        out=et[:].rearrange("p (b s) -> p b s", b=BPC),
                        in0=pe[:].rearrange("p (b s) -> p b s", b=BPC),
                        in1=uh[:, m, BPC * r : BPC * (r + 1)]
                        .unsqueeze(2)
                        .broadcast_to([128, BPC, S]),
                        op=OP.add,
                    )
                    tt = asb.tile([128, RC], F32R, tag="tt")
                    nc.scalar.activation(out=tt, in_=et, func=AF.Tanh)
                    nc.tensor.matmul(
                        ps_s[:],
                        vt[:, m : m + 1],
                        tt[:],
                        start=(m == 0),
                        stop=(m == NKT - 1),
                    )
                # scores PSUM -> SBUF, then scatter [1, 512] -> [2, 256] rows
                scc = asb.tile([1, RC], F32, tag="scc")
                nc.vector.tensor_copy(out=scc, in_=ps_s[:])
                nc.sync.dma_start(
                    out=sc[BPC * r : BPC * (r + 1), :],
                    in_=scc[:].rearrange("p (b s) -> p b s", b=BPC),
                )

        # softmax over S (alpha = softmax(sc + V_b) == softmax(sc))
        alpha = pp.tile([BL, S], F32, tag="alpha")
        with tc.tile_pool(name="soft", bufs=1) as sp:
            mx = sp.tile([BL, 1], F32, tag="mx")
            nc.vector.tensor_reduce(out=mx, in_=sc, axis=mybir.AxisListType.X, op=OP.max)
            mxn = sp.tile([BL, 1], F32, tag="mxn")
            nc.vector.tensor_scalar_mul(mxn[:], mx[:], -1.0)
            ex = sp.tile([BL, S], F32, tag="ex")
            sm = sp.tile([BL, 1], F32, tag="sm")
            nc.scalar.activation(
                out=ex, in_=sc, func=AF.Exp, bias=mxn[:], scale=1.0, accum_out=sm[:]
            )
            rs = sp.tile([BL, 1], F32, tag="rs")
            nc.vector.reciprocal(rs[:], sm[:])
            nc.vector.tensor_scalar_mul(alpha[:], ex[:], rs[:])

        # broadcast alpha to all partitions, then context^T via mul+reduce
        ctxT = pp.tile([128, NKT, BL], F32R, tag="ctxT")
        with (
            tc.tile_pool(name="ab_sb", bufs=1) as absb,
            tc.tile_pool(name="ctx_sb", bufs=2) as csb,
        ):
            nc.sync.dma_start(out=t["alpha_d"], in_=alpha[:])
            ab = absb.tile([128, BL, S], F32, tag="ab")
            nc.sync.dma_start(out=ab, in_=t["alpha_d"].partition_broadcast(128))
            for k in range(NKT):
                mt = csb.tile([128, BL, S], F32, tag="mt")
                nc.vector.tensor_tensor(
                    out=mt,
                    in0=enc[k][:].bitcast(F32).rearrange("p (b s) -> p b s", b=BL),
                    in1=ab[:],
                    op=OP.mult,
                )
                with nc.allow_low_precision(reason="float32r is 32-bit"):
                    nc.vector.tensor_reduce(
                        out=ctxT[:, k, :], in_=mt[:],
                        axis=mybir.AxisListType.X, op=OP.add,
                    )

        # ---------------- GRU (local batch rows) ----------------
        hn_row = pp.tile([BL, H], F32, tag="hn_row")
        with (
            tc.tile_pool(name="psG", bufs=1, space="PSUM") as psG,
            tc.tile_pool(name="gru_sb", bufs=1) as gsb,
        ):
            def xcat(k):  # lhsT k-tile of [embedded; context]^T
                return embt[:, k, :] if k < NKT else ctxT[:, k - NKT, :]

            def gi_chunk(ps_tile, j, include_ih=True, include_hh=True):
                """Accumulate gate chunk j (cols j*512:(j+1)*512 of 3H)."""
                first = True
                if include_ih:
                    bt_ = bs.tile([1, 512], F32R, tag="bih")
                    nc.sync.dma_start(out=bt_, in_=t["bih"][:, j * 512 : (j + 1) * 512])
                    nc.tensor.matmul(
                        ps_tile[:], onest[:, :BL], bt_[:], start=first, stop=False,
                    )
                    first = False
                if include_hh:
                    bt_ = bs.tile([1, 512], F32R, tag="bhh")
                    nc.sync.dma_start(out=bt_, in_=t["bhh"][:, j * 512 : (j + 1) * 512])
                    nc.tensor.matmul(
                        ps_tile[:], onest[:, :BL], bt_[:], start=first, stop=False,
                    )
                    first = False
                if include_ih:
                    for k in range(2 * NKT):
                        wt_ = ws.tile([128, 512], F32R, tag="wih")
                        nc.sync.dma_start(
                            out=wt_,
                            in_=t["wihT"][
                                k * 128 : (k + 1) * 128, j * 512 : (j + 1) * 512
                            ],
                        )
                        nc.tensor.matmul(
                            ps_tile[:], xcat(k), wt_[:],
                            start=False, stop=(not include_hh and k == 2 * NKT - 1),
                        )
                if include_hh:
                    for k in range(NKT):
                        wt_ = ws.tile([128, 512], F32R, tag="whh")
                        nc.sync.dma_start(
                            out=wt_,
                            in_=t["whhT"][
                                k * 128 : (k + 1) * 128, j * 512 : (j + 1) * 512
                            ],
                        )
                        nc.tensor.matmul(
                            ps_tile[:], hTt[:, k, :], wt_[:],
                            start=False, stop=(k == NKT - 1),
                        )

            prz0 = psG.tile([BL, 512], F32, tag="prz0")
            gi_chunk(prz0, 0)                          # r gate pre-activation
            prz1 = psG.tile([BL, 512], F32, tag="prz1")
            gi_chunk(prz1, 1)                          # z gate pre-activation
            pni = psG.tile([BL, 512], F32, tag="pni")
            gi_chunk(pni, 2, include_hh=False)         # i_n
            pnh = psG.tile([BL, 512], F32, tag="pnh")
            gi_chunk(pnh, 2, include_ih=False)         # h_n

            rg = gsb.tile([BL, 512], F32, tag="rg")
            nc.scalar.activation(out=rg, in_=prz0[:], func=AF.Sigmoid)
            zg = gsb.tile([BL, 512], F32, tag="zg")
            nc.scalar.activation(out=zg, in_=prz1[:], func=AF.Sigmoid)
            t1 = gsb.tile([BL, 512], F32, tag="t1")
            nc.vector.tensor_tensor(out=t1, in0=rg, in1=pnh[:], op=OP.mult)
            t2 = gsb.tile([BL, 512], F32, tag="t2")
            nc.vector.tensor_tensor(out=t2, in0=t1, in1=pni[:], op=OP.add)
            ng = gsb.tile([BL, 512], F32, tag="ng")
            nc.scalar.activation(out=ng, in_=t2, func=AF.Tanh)
            # h_new = n + z * (h - n)
            dg = gsb.tile([BL, 512], F32, tag="dg")
            nc.vector.tensor_tensor(out=dg, in0=hrow, in1=ng, op=OP.subtract)
            zd = gsb.tile([BL, 512], F32, tag="zd")
            nc.vector.tensor_tensor(out=zd, in0=zg, in1=dg, op=OP.mult)
            nc.vector.tensor_tensor(out=hn_row, in0=ng, in1=zd, op=OP.add)

        nc.sync.dma_start(out=t["hnew"], in_=hn_row[:])

        # transpose h_new -> [H, BL] and AllGather across cores
        hnT = pp.tile([128, NKT, BL], F32, tag="hnT")
        with tc.tile_pool(name="psT", bufs=2, space="PSUM") as psT:
            for k in range(NKT):
                ptr = psT.tile([128, BL], F32, tag="ptr")
                nc.tensor.transpose(
                    ptr[:], hn_row[:, k * 128 : (k + 1) * 128], id8[:]
                )
                nc.vector.tensor_copy(out=hnT[:, k, :], in_=ptr[:])
        nc.sync.dma_start(
            out=t["cc_in"].rearrange("(k p) b -> p k b", p=128), in_=hnT[:]
        )
        nc.gpsimd.collective_compute(
            "AllGather",
            OP.bypass,
            replica_groups=[list(range(NCORES))],
            ins=[t["cc_in"]],
            outs=[t["cc_out"]],
        )

        # ---------------- fc (vocab shard, full batch) ----------------
        with (
            tc.tile_pool(name="psF", bufs=3, space="PSUM") as psF,
            tc.tile_pool(name="fc_sb", bufs=3) as fsb,
        ):
            hTf = []
            for k in range(NKT):
                ht = fsb.tile([128, B], F32R, tag=f"hTf{k}")
                nc.gpsimd.dma_start(
                    out=ht,
                    in_=t["cc_out"].rearrange("c (k p) b -> k p c b", p=128)[k],
                )
                hTf.append(ht)
            for j in range(NFC):
                pf = psF.tile([B, FCN], F32, tag="pf")
                fb_ = bs.tile([1, FCN], F32R, tag="fcb")
                nc.sync.dma_start(out=fb_, in_=t["fcb"][:, j * FCN : (j + 1) * FCN])
                nc.tensor.matmul(
                    pf[:], onest[:, :B], fb_[:], start=True, stop=False,
                )
                for k in range(NKT):
                    ft = fs.tile([128, FCN], F32R, tag="fct")
                    nc.sync.dma_start(
                        out=ft,
                        in_=t["fcT"][k * 128 : (k + 1) * 128, j * FCN : (j + 1) * FCN],
                    )
                    nc.tensor.matmul(
                        pf[:], hTf[k][:], ft[:], start=False, stop=(k == NKT - 1)
                    )
                lg = fsb.tile([B, FCN], F32, tag="lg")
                nc.vector.tensor_copy(out=lg, in_=pf[:])
                nc.sync.dma_start(
                    out=t["logits"][:, j * FCN : (j + 1) * FCN], in_=lg[:]
                )


def build_program(unroll=1):
    nc = bacc.Bacc(
        "TRN2", target_bir_lowering=False, debug=False, num_devices=NCORES
    )
    t = _declare_io(nc)
    if unroll != 1:
        # Shape-varying dummy input: makes the HLO module signature unique per
        # unroll count so the NEFF compile cache cannot alias the variants.
        nc.dram_tensor("utick", [1, unroll], F32, kind="ExternalInput")
    with tile.TileContext(nc) as tc:
        for _ in range(unroll):
            _emit(nc, tc, t)
    nc.compile()
    return nc


def prepare_in_maps(inputs):
    """Shard + lay out the full inputs into 8 per-core input dicts."""
    f = np.float32
    x = np.asarray(inputs["x"])
    h = np.asarray(inputs["hidden"], dtype=f)[0]              # [B, H]
    enc = np.asarray(inputs["encoder_outputs"], dtype=f)      # [B, S, E]
    embedded = np.asarray(inputs["emb_table"], dtype=f)[x]    # [B, E]

    def C(a):
        return np.ascontiguousarray(a, dtype=f)

    shared = {
        "wwT": C(np.asarray(inputs["W_w"]).T),
        "uwT": C(np.asarray(inputs["U_w"]).T),
        "vw": C(np.asarray(inputs["V_w"]).reshape(1, A)),
        "wb": C(np.asarray(inputs["W_b"]).reshape(1, A)),
        "ub": C(np.asarray(inputs["U_b"]).reshape(1, A)),
        "wihT": C(np.asarray(inputs["W_ih"]).T),
        "whhT": C(np.asarray(inputs["W_hh"]).T),
        "bih": C(np.asarray(inputs["b_ih"]).reshape(1, G3)),
        "bhh": C(np.asarray(inputs["b_hh"]).reshape(1, G3)),
        "ones": np.ones((1, B), dtype=f),
        "id8": np.eye(BL, dtype=f),
    }
    fc_w = np.asarray(inputs["fc_w"], dtype=f)
    fc_b = np.asarray(inputs["fc_b"], dtype=f)

    in_maps = []
    for c in range(NCORES):
        bs = slice(c * BL, (c + 1) * BL)
        vs = slice(c * VS, (c + 1) * VS)
        m = dict(shared)
        m["encT"] = C(enc[bs].reshape(R, E).T)
        m["hT"] = C(h[bs].T)
        m["hrow"] = C(h[bs])
        m["embT"] = C(embedded[bs].T)
        m["fcT"] = C(fc_w[vs].T)
        m["fcb"] = C(fc_b[vs].reshape(1, VS))
        in_maps.append(m)
    return in_maps


def assemble(results):
    logits = np.concatenate([results[c]["logits"] for c in range(NCORES)], axis=1)
    hnew = np.concatenate([results[c]["hnew"] for c in range(NCORES)], axis=0)
    return logits, hnew[None]


_CACHED_NC = None


def kernel(**inputs):
    global _CACHED_NC
    from concourse.bass_utils import run_bass_kernel_spmd

    if _CACHED_NC is None:
        _CACHED_NC = build_program()
    in_maps = prepare_in_maps(inputs)
    res = run_bass_kernel_spmd(_CACHED_NC, in_maps, list(range(NCORES)))
    return assemble(res.results)
